# revision 4
# baseline (speedup 1.0000x reference)
"""Trainium2 kernel v3: gated-ramp histogram with 4-group-packed matmuls.

Same math as kernel2 (see its docstring): hist = within-block second
difference of C2[m,n] = sum_p [posH==m] * relu(posL - n), n = -1..31.

v3 packs GP=4 pixel-groups into one matmul: stationary = 4 groups' posH
one-hots (128 columns), moving = 4 groups' ramp planes (132 columns),
PSUM [128, 132]. Only the "diagonal" blocks (same group on both sides) are
read out; cross-group products land in never-read PSUM cells. This cuts PE
instruction count 4x (8192 matmuls instead of 32768), which matters on
hardware where per-instruction sync overhead serialized v2.

Column order: the weight AP traverses (m, g) with g innermost, so PE column
index = 4*m + g. The moving AP traverses (n, g) likewise: out column =
4*n + g. Diagonal block element: psum[4m+g, 4n+g] = C2_g[m, n].
"""

import sys

sys.path.insert(0, "/opt/trn_rl_repo")

import numpy as np

import concourse.mybir as mybir
import concourse.tile as tile_mod
from concourse import bass
from concourse.bass_utils import run_bass_kernel_spmd
from concourse.vector_clock import ScopedClock


def _split_drain_and_barrier(self, tick_clock, wait_clock):
    nc = self.nc
    collector = nc.sync.nop(nofuse=True, hint="drain_wait_split")
    wait_clock.add_sem_waits(
        collector.ins, ScopedClock({None: tick_clock.global_clock})
    )
    si = collector.ins.sync_info
    waits = list(si.on_wait) if si is not None else []
    if len(waits) > 1:
        collector.ins.sync_info = mybir.SyncInfo(
            on_wait=[waits[0]], on_update=list(si.on_update)
        )
        for w in waits[1:]:
            n = nc.sync.nop(nofuse=True, hint="drain_wait_split")
            n.ins.sync_info = mybir.SyncInfo(on_wait=[w], on_update=[])

    nc.sync.drain()
    nc.all_engine_barrier()
    assert self.sems is not None
    popped = nc._tile_sem_poison_stack.pop()
    assert popped is self._sem_poison
    nc.clear_and_free_semaphores(list(self.sems.allocated().values()))
    nc.all_engine_barrier()


tile_mod.TileContext._drain_and_barrier = _split_drain_and_barrier

B = 16
H = W = 1024
NCORES = 8
B_PER_CORE = B // NCORES
NSLAB = B_PER_CORE * 2 * 4          # 16 slabs per core
PH, PW = H // 2, W // 2
NPIX = PH * PW                      # 262144 pixels per slab
P = 128
FREE = NPIX // P                    # 2048 pixel-groups per slab
CHUNK = 512                         # groups per encoding tile
MH = 32                             # hi-radix
NR = 33                             # ramps n = -1..31
GP = 4                              # groups packed per matmul
N_BINS = 1024
N_ACT = 15                          # ramp planes on the Scalar engine

f32 = mybir.dt.float32
f16 = mybir.dt.float16
A = mybir.AluOpType

_program_cache = {}


def _build_program():
    nc = bass.Bass()
    for n in range(32 - N_ACT, 32):
        val = -float(n)
        t = nc.alloc_sbuf_tensor(f"const-ramp-{n}", [128, 1], f32)
        nc.gpsimd.memset(t.ap(), val)
        nc.const_aps.aps[(f32, val)] = t.ap()
    nc.all_engine_barrier()
    x_d = nc.declare_dram_parameter("x", [NSLAB, 2, P, FREE], f16, isOutput=False)
    out_d = nc.declare_dram_parameter(
        "out", [NSLAB, MH * GP, NR * GP], f32, isOutput=True
    )

    with tile_mod.TileContext(nc) as tc:
        with (
            tc.tile_pool(name="xin", bufs=2) as xpool,
            tc.tile_pool(name="enc", bufs=2) as epool,
            tc.tile_pool(name="psum", bufs=2, space="PSUM") as ppool,
            tc.tile_pool(name="outp", bufs=2) as opool,
        ):
            for s in range(NSLAB):
                xh = xpool.tile([P, FREE], f16, tag="xh")
                xl = xpool.tile([P, FREE], f16, tag="xl")
                nc.sync.dma_start(xh[:], x_d[s, 0])
                nc.sync.dma_start(xl[:], x_d[s, 1])

                psum_t = ppool.tile([MH * GP, NR * GP], f32, tag="acc")
                for c0 in range(0, FREE, CHUNK):
                    cs = slice(c0, c0 + CHUNK)
                    # block-contiguous layout: [p, group-of-GP, plane, g]
                    # so each matmul's 128/132 columns are one contiguous run
                    enc = epool.tile([P, CHUNK // GP, MH + NR, GP], f16, tag="enc")
                    xh3 = xh[:, cs].rearrange("p (cb g) -> p cb g", g=GP)
                    xl3 = xl[:, cs].rearrange("p (cb g) -> p cb g", g=GP)
                    for m in range(MH):
                        nc.vector.tensor_scalar(
                            out=enc[:, :, m, :], in0=xh3,
                            scalar1=float(m), scalar2=None, op0=A.is_equal,
                        )
                    nc.vector.tensor_scalar(
                        out=enc[:, :, MH, :], in0=xl3,
                        scalar1=1.0, scalar2=None, op0=A.add,
                    )
                    for n in range(32):
                        if n >= 32 - N_ACT:
                            nc.scalar.activation(
                                enc[:, :, MH + 1 + n, :], xl3,
                                mybir.ActivationFunctionType.Relu,
                                bias=-float(n), scale=1.0,
                            )
                        else:
                            nc.vector.tensor_scalar(
                                out=enc[:, :, MH + 1 + n, :], in0=xl3,
                                scalar1=float(n), scalar2=0.0,
                                op0=A.subtract, op1=A.max,
                            )
                    for cb in range(CHUNK // GP):
                        gi = c0 + cb * GP
                        nc.tensor.matmul(
                            out=psum_t[:],
                            lhsT=enc[:, cb, 0:MH, :].rearrange("p a b -> p (a b)"),
                            rhs=enc[:, cb, MH:, :].rearrange("p a b -> p (a b)"),
                            start=(gi == 0),
                            stop=(gi == FREE - GP),
                        )

                out_s = opool.tile([MH * GP, NR * GP], f32, tag="o")
                nc.scalar.copy(out_s[:], psum_t[:])
                nc.sync.dma_start(out_d[s], out_s[:])

    import bass_rust  # noqa: PLC0415

    bass_rust.generate_event_semaphores(nc)
    return nc


def _get_program():
    if "nc" not in _program_cache:
        _program_cache["nc"] = _build_program()
    return _program_cache["nc"]


def _prep_core_input(gt, ot, c):
    slabs = []
    for bl in range(B_PER_CORE):
        b = B_PER_CORE * c + bl
        for arr in (gt, ot):
            for i in (0, 1):
                for j in (0, 1):
                    slabs.append(arr[b, 0, i::2, j::2])
    x = np.stack(slabs).astype(np.float64).reshape(NSLAB, P, FREE)
    posH = np.floor(x / 32.0)
    posL = x - 32.0 * posH
    planes = np.empty((NSLAB, 2, P, FREE), np.float16)
    planes[:, 0] = posH.astype(np.float16)
    planes[:, 1] = posL.astype(np.float16)
    return planes


def _hist_from_c2(c2):
    """c2: [NSLAB, MH, NR] float64 -> hist [NSLAB, N_BINS] (unnormalized)."""
    r = np.zeros((c2.shape[0], MH, NR + 1))
    r[:, :, :NR] = c2
    hist = r[:, :, 0:32] - 2.0 * r[:, :, 1:33] + r[:, :, 2:34]
    hist[:, 1:, 0] += c2[:, :-1, 32]
    return hist.reshape(c2.shape[0], N_BINS)


def kernel(bayer_gt: np.ndarray, bayer_out: np.ndarray) -> np.ndarray:
    gt = np.asarray(bayer_gt, dtype=np.float32)
    ot = np.asarray(bayer_out, dtype=np.float32)

    in_maps = [{"x": _prep_core_input(gt, ot, c)} for c in range(NCORES)]

    nc = _get_program()
    import os  # noqa: PLC0415

    trace = bool(os.environ.get("KL_TRACE"))
    res = None
    for attempt in range(3):
        try:
            res = run_bass_kernel_spmd(
                nc, in_maps, list(range(NCORES)), trace=trace
            )
            break
        except Exception:
            if attempt == 2:
                raise
    assert res is not None
    _program_cache["last_results"] = res

    n = float(NPIX)
    kl_per_phase = np.zeros(4, dtype=np.float64)
    for c in range(NCORES):
        raw = np.asarray(res.results[c]["out"], dtype=np.float64)
        raw = raw.reshape(NSLAB, MH, GP, NR, GP)
        # diagonal blocks: psum[4m+g, 4n+g] = C2_g[m, n]; sum the 4 subsets
        c2 = np.einsum("smgng->smn", raw)
        h = _hist_from_c2(c2) / n
        h = np.where(h != 0.0, h, 1.0 / n)
        lh = np.log(np.maximum(h, 1e-300))
        for bl in range(B_PER_CORE):
            for p in range(4):
                sg = bl * 8 + p
                so = bl * 8 + 4 + p
                hg, ho = h[sg], h[so]
                lg, lo = lh[sg], lh[so]
                kl = 0.5 * (np.sum(hg * (lg - lo)) + np.sum(ho * (lo - lg)))
                kl_per_phase[p] += kl

    return np.float32(kl_per_phase.mean())
